# revision 2
# baseline (speedup 1.0000x reference)
"""Trainium2 Bass kernel for the 4-layer sum/product circuit
(nn_KnowledgeLayer): h = enc(x); h = h[idx0].prod(1); h = h[idx1].sum(1);
h = h[idx2].prod(1); h = h[idx3].sum(1).

Strategy (slot-sharded, v2):
  * Host composes the four index maps into two operand streams A/B of
    32768 table indices (8 h0-slots per final output, depth-first), and
    builds the 4098-row encode table enc = [x | 1-x | 0 | 1] full-batch
    (1024 cols).
  * Each of the 8 cores computes 512 of the 4096 output rows over the
    FULL batch: it dma_gathers 2 x 4096 rows of 4 KB each from enc in
    DRAM (vs 2 x 32768 rows of 512 B for batch sharding - 8x fewer,
    8x fatter descriptors), then DVE tree-reduces mul/add/mul/add and
    DMAs its [512, 1024] slice out.  Outputs concatenate on axis 0.
"""

import numpy as np

N_VARS = 2048
BATCH = 1024
NCORES = 8
TABLE_ROWS = 2 * N_VARS + 2      # 4098
NOUT = 4096                      # h3 rows total
CORE_OUT = NOUT // NCORES        # 512 output rows per core
NCHUNK = CORE_OUT // 128         # 4 chunks of 128 outputs
GI = 1024                        # indices per gather call (128 outs x 8 slots)


# ----------------------------------------------------------------------------
# NTFF profile hook shim: this image's `antenv` lacks `axon_hooks`, which
# bass_utils imports unconditionally when trace=True under axon.  Provide the
# module and register the ctypes-based hook from trn_agent_boot.
# ----------------------------------------------------------------------------

def _ensure_ntff_hook():
    import sys
    try:
        from antenv import axon_hooks  # noqa: F401
        return
    except ImportError:
        pass
    import types
    mod = types.ModuleType("antenv.axon_hooks")
    mod._hook = None

    def set_axon_ntff_profile_hook(h):
        mod._hook = h

    def get_axon_ntff_profile_hook():
        return mod._hook

    mod.set_axon_ntff_profile_hook = set_axon_ntff_profile_hook
    mod.get_axon_ntff_profile_hook = get_axon_ntff_profile_hook
    sys.modules["antenv.axon_hooks"] = mod
    try:
        import antenv
        antenv.axon_hooks = mod
    except ImportError:
        pass
    try:
        from trn_agent_boot.trn_boot import _ntff_profile_via_ctypes
        hook = _ntff_profile_via_ctypes("/opt/axon/libaxon_pjrt.so")
        if hook is not None:
            mod._hook = hook
    except Exception:
        pass


try:
    _ensure_ntff_hook()
except Exception:
    pass


# ----------------------------------------------------------------------------
# host-side index preparation
# ----------------------------------------------------------------------------

def _compose_indices(idx0, idx1, idx2, idx3):
    """Return S_A, S_B: [4096, 8] table indices (slot k of output o)."""
    J = idx3.reshape(-1)
    K = idx2[J].reshape(-1)
    L = idx1[K].reshape(-1)
    AB = idx0[L]
    A, B = AB[:, 0].astype(np.int64), AB[:, 1].astype(np.int64)

    def remap(e):
        out = np.empty_like(e)
        out[e == 0] = 2 * N_VARS
        out[e == 1] = 2 * N_VARS + 1
        even = (e >= 2) & (e % 2 == 0)
        out[even] = (e[even] - 2) // 2
        odd = (e >= 3) & (e % 2 == 1)
        out[odd] = N_VARS + (e[odd] - 3) // 2
        return out

    return remap(A).reshape(NOUT, 8), remap(B).reshape(NOUT, 8)


def _wrap_core_idx(S, c):
    """Build the wrapped SWDGE index tensor [128, NCHUNK*GI//16] int16 for
    core c.  Chunk cc columns [cc*64,(cc+1)*64): gather element i fetches
    S[c*512 + cc*128 + i%128, i//128]."""
    cols = []
    for cc in range(NCHUNK):
        base = c * CORE_OUT + cc * 128
        blk = S[base:base + 128, :]          # [128 outs, 8 slots]
        idx_call = blk.T.reshape(-1)         # element i = blk[i%128, i//128]
        w = idx_call.reshape(-1, 16).T.astype(np.int16)   # [16, 64]
        cols.append(np.tile(w, (8, 1)))      # [128, 64]
    return np.ascontiguousarray(np.concatenate(cols, axis=1))


# ----------------------------------------------------------------------------
# bass program (built once, cached)
# ----------------------------------------------------------------------------

_CACHED = {}


def _build_program():
    import concourse.bacc as bacc
    import concourse.mybir as mybir
    from concourse.tile import TileContext

    f32 = mybir.dt.float32
    i16 = mybir.dt.int16

    nc = bacc.Bacc("TRN2", target_bir_lowering=False, debug=False)

    enc = nc.dram_tensor("enc", [TABLE_ROWS, BATCH], f32, kind="ExternalInput")
    idxa = nc.dram_tensor("idxa", [128, NCHUNK * GI // 16], i16,
                          kind="ExternalInput")
    idxb = nc.dram_tensor("idxb", [128, NCHUNK * GI // 16], i16,
                          kind="ExternalInput")
    out = nc.dram_tensor("out", [CORE_OUT, BATCH], f32, kind="ExternalOutput")

    with TileContext(nc) as tc:
        with tc.tile_pool(name="setup", bufs=1) as sp, \
             tc.tile_pool(name="gather", bufs=2) as gp, \
             tc.tile_pool(name="mid", bufs=1) as mp, \
             tc.tile_pool(name="outp", bufs=2) as op:

            ia = sp.tile([128, NCHUNK * GI // 16], i16, tag="ia")
            ib = sp.tile([128, NCHUNK * GI // 16], i16, tag="ib")
            nc.sync.dma_start(out=ia[:, :], in_=idxa[:, :])
            nc.sync.dma_start(out=ib[:, :], in_=idxb[:, :])

            for cc in range(NCHUNK):
                ga = gp.tile([128, 8, BATCH], f32, tag="ga")
                gb = gp.tile([128, 8, BATCH], f32, tag="gb")
                c0 = cc * (GI // 16)
                nc.gpsimd.dma_gather(
                    out_ap=ga[:, :, :],
                    in_ap=enc[:, :],
                    idxs_ap=ia[:, c0:c0 + GI // 16],
                    num_idxs=GI, num_idxs_reg=GI,
                    elem_size=BATCH,
                    single_packet=False)
                nc.gpsimd.dma_gather(
                    out_ap=gb[:, :, :],
                    in_ap=enc[:, :],
                    idxs_ap=ib[:, c0:c0 + GI // 16],
                    num_idxs=GI, num_idxs_reg=GI,
                    elem_size=BATCH,
                    single_packet=False)

                # h0[k] = ga[k]*gb[k]; h1[m] = h0[2m]+h0[2m+1];
                # h2[a] = h1[2a]*h1[2a+1]; h3 = h2[0]+h2[1]
                e0 = mp.tile([128, 4, BATCH], f32, tag="e0")
                nc.vector.tensor_mul(
                    e0[:, :, :], ga[:, 0:8:2, :], gb[:, 0:8:2, :])
                e1 = mp.tile([128, 4, BATCH], f32, tag="e1")
                nc.vector.tensor_mul(
                    e1[:, :, :], ga[:, 1:8:2, :], gb[:, 1:8:2, :])
                h1 = mp.tile([128, 4, BATCH], f32, tag="h1")
                nc.vector.tensor_add(h1[:, :, :], e0[:, :, :], e1[:, :, :])
                h2 = mp.tile([128, 2, BATCH], f32, tag="h2")
                nc.vector.tensor_mul(
                    h2[:, :, :], h1[:, 0:4:2, :], h1[:, 1:4:2, :])
                h3 = op.tile([128, 1, BATCH], f32, tag="h3")
                nc.vector.tensor_add(
                    h3[:, :, :], h2[:, 0:1, :], h2[:, 1:2, :])

                nc.sync.dma_start(
                    out=out[cc * 128:(cc + 1) * 128, :]
                        .rearrange("(k p) f -> p k f", p=128),
                    in_=h3[:, :, :])

    nc.compile()
    return nc


def _get_program():
    if "nc" not in _CACHED:
        _CACHED["nc"] = _build_program()
    return _CACHED["nc"]


# ----------------------------------------------------------------------------
# public entry point
# ----------------------------------------------------------------------------

def kernel(x, idx0, idx1, idx2, idx3, _trace=False, _trace_kwargs=None):
    from concourse.bass_utils import run_bass_kernel_spmd

    x = np.asarray(x, dtype=np.float32)
    enc_np = np.empty((TABLE_ROWS, BATCH), np.float32)
    enc_np[0:N_VARS] = x
    enc_np[N_VARS:2 * N_VARS] = 1.0 - x
    enc_np[2 * N_VARS] = 0.0
    enc_np[2 * N_VARS + 1] = 1.0
    S_A, S_B = _compose_indices(
        np.asarray(idx0), np.asarray(idx1), np.asarray(idx2), np.asarray(idx3))

    nc = _get_program()
    in_maps = []
    for c in range(NCORES):
        in_maps.append({
            "enc": enc_np,
            "idxa": _wrap_core_idx(S_A, c),
            "idxb": _wrap_core_idx(S_B, c),
        })

    kwargs = {}
    if _trace:
        kwargs["trace"] = True
        if _trace_kwargs:
            kwargs.update(_trace_kwargs)
    res = run_bass_kernel_spmd(nc, in_maps, core_ids=list(range(NCORES)), **kwargs)
    outs = [res.results[c]["out"] for c in range(NCORES)]
    full = np.concatenate(outs, axis=0)
    if _trace:
        kernel.last_exec_time_ns = res.exec_time_ns
        kernel.last_profile = res.profile_json
    return full
